# revision 6
# baseline (speedup 1.0000x reference)
"""Multi-head self-attention (no causal mask) on 8 Trainium2 NeuronCores.

Problem: B=2, S=2048, D=768, H=12 heads (head_dim 64), fp32.
Sharding: batch x head-group. Core c handles batch c//4 and heads
3*(c%4) .. 3*(c%4)+2 (Megatron column-parallel QKV, row-parallel Wo).
Each core computes a partial [2048, 768] output (its heads' contribution
through Wo); the host sums the 4 partials per batch and adds bo.

Per-core kernel outline (all fp32):
  - load x[b] [2048,768]; transpose on PE to x^T [768,2048] in SBUF
  - Q^T/K^T per head in [64, 2048] layout (scale 1/8 + bias folded in),
    V in natural [2048, 64] layout with a ones-column appended
  - per (head, q-half, k-tile): scores^T = K^T.T @ Q^T chunks -> PSUM,
    exp on ScalarE -> p^T in SBUF, attn@V accumulates [V|1].T @ p^T
    giving both the unnormalized output and the softmax denominators
  - normalize by broadcasted reciprocal sums, out-project with Wo
"""

import numpy as np

_CACHE = {}

S = 2048
D = 768
HLOC = 3          # heads per core
NKT = 6           # 768 / 128 d-tiles
NST = 16          # 2048 / 128 s-tiles
# head -> (slot, base partition) in the Q^T/K^T storage tiles
HPOS = [(0, 0), (0, 64), (1, 0)]


def _emit(nc, tc, ctx, dram, loop_n=None):
    import concourse.bass as bass
    import concourse.mybir as mybir
    from concourse.masks import make_identity

    f32 = mybir.dt.float32
    add = mybir.AluOpType.add
    mult = mybir.AluOpType.mult
    Exp = mybir.ActivationFunctionType.Exp

    xb, wq, wk, wv, wo, bq, bk, bv, out_d = (
        dram["xb"], dram["wq"], dram["wk"], dram["wv"], dram["wo"],
        dram["bq"], dram["bk"], dram["bv"], dram["out"],
    )

    consts = ctx.enter_context(tc.tile_pool(name="consts", bufs=1))
    xpool = ctx.enter_context(tc.tile_pool(name="xpool", bufs=5))
    ppool = ctx.enter_context(tc.tile_pool(name="ppool", bufs=6))
    accpool = ctx.enter_context(tc.tile_pool(name="accpool", bufs=3))
    rpool = ctx.enter_context(tc.tile_pool(name="rpool", bufs=2))
    bpool = ctx.enter_context(tc.tile_pool(name="bpool", bufs=2))
    opool = ctx.enter_context(tc.tile_pool(name="opool", bufs=3))
    pab = ctx.enter_context(tc.tile_pool(name="pab", bufs=2, space="PSUM"))
    pspool = ctx.enter_context(tc.tile_pool(name="pspool", bufs=2, space="PSUM"))
    popool = ctx.enter_context(tc.tile_pool(name="popool", bufs=2, space="PSUM"))

    # ---- constants / persistent tensors ----
    ident = consts.tile([128, 128], f32)
    make_identity(nc, ident)

    xt = consts.tile([128, NKT, S], f32)           # x^T
    qt = consts.tile([128, 2, S], f32)             # Q^T: slot0=[A;B], slot1=[C;-]
    kt_ = consts.tile([128, 2, S], f32)            # K^T likewise
    v_sb = consts.tile([128, NST, HLOC, 72], f32)  # V natural + ones col at 64
    attnT_AB = consts.tile([128, S], f32)          # normalized attn^T heads A,B
    attnT_C = consts.tile([64, S], f32)            # head C

    w_qsb = consts.tile([128, NKT, 192], f32)
    w_ksb = consts.tile([128, NKT, 192], f32)
    w_vsb = consts.tile([128, NKT, 192], f32)
    w_osb = consts.tile([128, 2, D], f32)
    bq1 = consts.tile([128, 1], f32)
    bq2 = consts.tile([64, 1], f32)
    bk1 = consts.tile([128, 1], f32)
    bk2 = consts.tile([64, 1], f32)
    bv_bc = consts.tile([128, 192], f32)

    # ---- weight / bias loads (outside any timing loop) ----
    nc.sync.dma_start(out=w_qsb, in_=wq.rearrange("(t p) c -> p t c", p=128))
    nc.sync.dma_start(out=w_ksb, in_=wk.rearrange("(t p) c -> p t c", p=128))
    nc.sync.dma_start(out=w_vsb, in_=wv.rearrange("(t p) c -> p t c", p=128))
    nc.sync.dma_start(out=w_osb[:, 0, :], in_=wo[0:128, :])
    nc.sync.dma_start(out=w_osb[0:64, 1, :], in_=wo[128:192, :])
    nc.sync.dma_start(out=bq1, in_=bq[0:128].rearrange("(p o) -> p o", o=1))
    nc.sync.dma_start(out=bq2, in_=bq[128:192].rearrange("(p o) -> p o", o=1))
    nc.sync.dma_start(out=bk1, in_=bk[0:128].rearrange("(p o) -> p o", o=1))
    nc.sync.dma_start(out=bk2, in_=bk[128:192].rearrange("(p o) -> p o", o=1))
    bv_b = bass.AP(tensor=bv.tensor, offset=bv.offset, ap=[[0, 128]] + list(bv.ap))
    nc.sync.dma_start(out=bv_bc, in_=bv_b)
    nc.vector.memset(v_sb[:, :, :, 64:65], 1.0)

    acc_tiles = {}

    def qsl(g):
        return slice(g * 512, (g + 1) * 512)

    def emit_attn_block(qh, blk):
        """scores + exp + attn@V for k-tiles 4*blk..4*blk+3 of q-half qh."""
        for h in range(HLOC):
            slot, base = HPOS[h]
            if blk == 0:
                acc_tiles[(h, qh)] = accpool.tile([65, 1024], f32, name=f"acc_{h}_{qh}", tag="acc")
            acc = acc_tiles[(h, qh)]
            ptiles = []
            for kti in range(4 * blk, 4 * blk + 4):
                ps = pspool.tile([128, 1024], f32, name=f"ps_{h}_{qh}_{kti}", tag="ps")
                for c in range(2):
                    nc.tensor.matmul(
                        ps[:, c * 512:(c + 1) * 512],
                        lhsT=kt_[base:base + 64, slot, kti * 128:(kti + 1) * 128],
                        rhs=qt[base:base + 64, slot,
                               qh * 1024 + c * 512: qh * 1024 + (c + 1) * 512],
                        start=True, stop=True)
                p_t = ppool.tile([128, 1024], f32, name=f"p_{h}_{qh}_{kti}", tag="p")
                nc.scalar.activation(out=p_t, in_=ps, func=Exp)
                ptiles.append((kti, p_t))
            for c in range(2):
                po = popool.tile([65, 512], f32, name=f"po_{h}_{qh}_{blk}_{c}", tag="po")
                for i, (kti, p_t) in enumerate(ptiles):
                    nc.tensor.matmul(
                        po,
                        lhsT=v_sb[:, kti, h, 0:65],
                        rhs=p_t[:, c * 512:(c + 1) * 512],
                        start=(i == 0), stop=(i == 3))
                dst = acc[:, c * 512:(c + 1) * 512]
                if blk == 0:
                    nc.vector.tensor_copy(out=dst, in_=po)
                else:
                    nc.vector.tensor_tensor(out=dst, in0=dst, in1=po, op=add)

    def finish_qh(qh):
        qhs = slice(qh * 1024, (qh + 1) * 1024)
        for h in range(HLOC):
            acc = acc_tiles.pop((h, qh))
            r_t = rpool.tile([1, 1024], f32, name=f"r_{h}_{qh}", tag="r")
            nc.vector.reciprocal(out=r_t, in_=acc[64:65, :])
            b_t = bpool.tile([64, 1024], f32, name=f"b_{h}_{qh}", tag="b")
            nc.gpsimd.partition_broadcast(b_t, r_t)
            if h == 0:
                dst = attnT_AB[0:64, qhs]
            elif h == 1:
                dst = attnT_AB[64:128, qhs]
            else:
                dst = attnT_C[0:64, qhs]
            nc.vector.tensor_tensor(out=dst, in0=acc[0:64, :], in1=b_t, op=mult)
        for sti in range(qh * 8, qh * 8 + 8):
            ssl = slice(sti * 128, (sti + 1) * 128)
            o_t = opool.tile([128, D], f32, name=f"o_{sti}", tag="o")
            for e in range(2):
                esl = slice(e * 384, (e + 1) * 384)
                pw = pab.tile([128, 512], f32, tag="mm", name=f"pw_{sti}_{e}")
                nc.tensor.matmul(pw[:, 0:384], lhsT=attnT_AB[:, ssl],
                                 rhs=w_osb[:, 0, esl], start=True, stop=False)
                nc.tensor.matmul(pw[:, 0:384], lhsT=attnT_C[0:64, ssl],
                                 rhs=w_osb[0:64, 1, esl], start=False, stop=True)
                nc.vector.tensor_copy(out=o_t[:, esl], in_=pw[:, 0:384])
            nc.sync.dma_start(out=out_d[ssl, :], in_=o_t)

    def body():
        # main pipeline over s-tile groups of 4 (one q-chunk of 512 each)
        for g in range(4):
            xg = []
            for j in range(4):
                st = 4 * g + j
                x_t = xpool.tile([128, D], f32, name=f"x_{st}", tag="x")
                nc.sync.dma_start(out=x_t, in_=xb[st * 128:(st + 1) * 128, :])
                xg.append(x_t)
            for dt in range(NKT):
                pt = pab.tile([128, 512], f32, tag="mm", name=f"pt_{g}_{dt}")
                for j in range(4):
                    nc.tensor.transpose(pt[:, j * 128:(j + 1) * 128],
                                        xg[j][:, dt * 128:(dt + 1) * 128], ident)
                nc.vector.tensor_copy(out=xt[:, dt, qsl(g)], in_=pt)
            # Q/K projections for q-chunk g
            for dst, wsb, b1, b2, sc in ((qt, w_qsb, bq1, bq2, 0.125),
                                         (kt_, w_ksb, bk1, bk2, None)):
                pp = pab.tile([128, 512], f32, tag="mm", name=f"pp_{g}")
                for kti in range(NKT):
                    nc.tensor.matmul(pp, lhsT=wsb[:, kti, 0:128],
                                     rhs=xt[:, kti, qsl(g)],
                                     start=(kti == 0), stop=(kti == NKT - 1))
                if sc is None:
                    nc.vector.tensor_scalar_add(dst[:, 0, qsl(g)], pp, b1)
                else:
                    nc.vector.tensor_scalar(dst[:, 0, qsl(g)], pp, b1, sc,
                                            add, mult)
                pp2 = pab.tile([128, 512], f32, tag="mm", name=f"pp2_{g}")
                for kti in range(NKT):
                    nc.tensor.matmul(pp2[0:64, :], lhsT=wsb[:, kti, 128:192],
                                     rhs=xt[:, kti, qsl(g)],
                                     start=(kti == 0), stop=(kti == NKT - 1))
                if sc is None:
                    nc.vector.tensor_scalar_add(dst[0:64, 1, qsl(g)],
                                                pp2[0:64, :], b2)
                else:
                    nc.vector.tensor_scalar(dst[0:64, 1, qsl(g)], pp2[0:64, :],
                                            b2, sc, add, mult)
            # V projection for s-tiles in group g
            for j in range(4):
                st = 4 * g + j
                pv = pab.tile([128, 512], f32, tag="mm", name=f"pv_{st}")
                for kti in range(NKT):
                    nc.tensor.matmul(pv[:, 0:192],
                                     lhsT=xt[:, kti, st * 128:(st + 1) * 128],
                                     rhs=w_vsb[:, kti, :],
                                     start=(kti == 0), stop=(kti == NKT - 1))
                nc.vector.tensor_tensor(
                    out=v_sb[:, st, :, 0:64],
                    in0=pv[:, 0:192].rearrange("p (h d) -> p h d", h=3),
                    in1=bv_bc.rearrange("p (h d) -> p h d", h=3),
                    op=add)
            # attention work unlocked by this group
            if g == 1:
                emit_attn_block(0, 0)
                emit_attn_block(0, 1)
            elif g == 2:
                emit_attn_block(0, 2)
            elif g == 3:
                emit_attn_block(0, 3)
                finish_qh(0)
                for blk in range(4):
                    emit_attn_block(1, blk)
                finish_qh(1)

    if loop_n is None:
        body()
    else:
        with tc.For_i(0, loop_n, 1):
            body()


def _build(loop_n=None):
    from contextlib import ExitStack

    import concourse.bacc as bacc
    import concourse.mybir as mybir
    import concourse.tile as tile

    f32 = mybir.dt.float32
    nc = bacc.Bacc("TRN2", target_bir_lowering=False, debug=False, num_devices=8)
    dram = {
        "xb": nc.dram_tensor("xb", [S, D], f32, kind="ExternalInput").ap(),
        "wq": nc.dram_tensor("wq", [D, 192], f32, kind="ExternalInput").ap(),
        "wk": nc.dram_tensor("wk", [D, 192], f32, kind="ExternalInput").ap(),
        "wv": nc.dram_tensor("wv", [D, 192], f32, kind="ExternalInput").ap(),
        "wo": nc.dram_tensor("wo", [192, D], f32, kind="ExternalInput").ap(),
        "bq": nc.dram_tensor("bq", [192], f32, kind="ExternalInput").ap(),
        "bk": nc.dram_tensor("bk", [192], f32, kind="ExternalInput").ap(),
        "bv": nc.dram_tensor("bv", [192], f32, kind="ExternalInput").ap(),
        "out": nc.dram_tensor("out", [S, D], f32, kind="ExternalOutput").ap(),
    }
    with tile.TileContext(nc) as tc:
        with ExitStack() as ctx:
            _emit(nc, tc, ctx, dram, loop_n=loop_n)
    nc.compile()
    return nc


def _get_nc():
    if "nc" not in _CACHE:
        _CACHE["nc"] = _build()
    return _CACHE["nc"]


def _shard(inputs):
    x = np.asarray(inputs["x"], np.float32)
    Wq = np.asarray(inputs["Wq"], np.float32)
    Wk = np.asarray(inputs["Wk"], np.float32)
    Wv = np.asarray(inputs["Wv"], np.float32)
    Wo = np.asarray(inputs["Wo"], np.float32)
    bq = np.asarray(inputs["bq"], np.float32)
    bk = np.asarray(inputs["bk"], np.float32)
    bv = np.asarray(inputs["bv"], np.float32)
    in_maps = []
    for c in range(8):
        b, g = divmod(c, 4)
        o = 192 * g
        in_maps.append({
            "xb": np.ascontiguousarray(x[b]),
            "wq": np.ascontiguousarray(Wq[:, o:o + 192]),
            "wk": np.ascontiguousarray(Wk[:, o:o + 192]),
            "wv": np.ascontiguousarray(Wv[:, o:o + 192]),
            "wo": np.ascontiguousarray(Wo[o:o + 192, :]),
            "bq": np.ascontiguousarray(bq[o:o + 192]),
            "bk": np.ascontiguousarray(bk[o:o + 192]),
            "bv": np.ascontiguousarray(bv[o:o + 192]),
        })
    return in_maps


def kernel(x, Wq, bq, Wk, bk, Wv, bv, Wo, bo):
    from concourse.bass_utils import run_bass_kernel_spmd

    nc = _get_nc()
    in_maps = _shard(dict(x=x, Wq=Wq, Wk=Wk, Wv=Wv, Wo=Wo,
                          bq=bq, bk=bk, bv=bv))
    res = run_bass_kernel_spmd(nc, in_maps, core_ids=list(range(8)))
    out = np.zeros((2, S, D), np.float32)
    for c in range(8):
        out[c // 4] += res.results[c]["out"]
    out += np.asarray(bo, np.float32)
    return out


# revision 12
# speedup vs baseline: 2.6007x; 2.6007x over previous
"""Multi-head self-attention (no causal mask) on 8 Trainium2 NeuronCores.

Problem: B=2, S=2048, D=768, H=12 heads (head_dim 64), fp32.
Sharding: batch x head-group. Core c handles batch c//4 and heads
3*(c%4) .. 3*(c%4)+2 (Megatron column-parallel QKV, row-parallel Wo).
Each core computes a partial [2048, 768] output (its heads' contribution
through Wo); the host sums the 4 partials per batch and adds bo.

Per-core kernel outline (all fp32):
  - load x[b] [2048,768]; transpose on PE to x^T [768,2048] in SBUF
  - Q^T/K^T per head in [64, 2048] layout (scale 1/8 + bias folded in),
    V in natural [2048, 64] layout with a ones-column appended
  - per (head, q-half, k-tile): scores^T = K^T.T @ Q^T chunks -> PSUM,
    exp on ScalarE -> p^T in SBUF, attn@V accumulates [V|1].T @ p^T
    giving both the unnormalized output and the softmax denominators
  - normalize by broadcasted reciprocal sums, out-project with Wo
"""

import numpy as np

_CACHE = {}

S = 2048
D = 768
HLOC = 3          # heads per core
NKT = 6           # 768 / 128 d-tiles
NST = 16          # 2048 / 128 s-tiles
# head -> (slot, base partition) in the Q^T/K^T storage tiles
HPOS = [(0, 0), (0, 64), (1, 0)]


def _emit(nc, tc, ctx, dram, loop_n=None, phase=4):
    import concourse.bass as bass
    import concourse.mybir as mybir
    from concourse.masks import make_identity

    f32 = mybir.dt.float32
    f32r = mybir.dt.float32r
    add = mybir.AluOpType.add
    mult = mybir.AluOpType.mult
    Exp = mybir.ActivationFunctionType.Exp

    xb, wq, wk, wv, wo, bq, bk, bv, out_d = (
        dram["xb"], dram["wq"], dram["wk"], dram["wv"], dram["wo"],
        dram["bq"], dram["bk"], dram["bv"], dram["out"],
    )

    consts = ctx.enter_context(tc.tile_pool(name="consts", bufs=1))
    xpool = ctx.enter_context(tc.tile_pool(name="xpool", bufs=5))
    ppool = ctx.enter_context(tc.tile_pool(name="ppool", bufs=8))
    accpool = ctx.enter_context(tc.tile_pool(name="accpool", bufs=3))
    rpool = ctx.enter_context(tc.tile_pool(name="rpool", bufs=1))
    bpool = ctx.enter_context(tc.tile_pool(name="bpool", bufs=1))
    opool = ctx.enter_context(tc.tile_pool(name="opool", bufs=2))
    pab = ctx.enter_context(tc.tile_pool(name="pab", bufs=2, space="PSUM"))
    pspool = ctx.enter_context(tc.tile_pool(name="pspool", bufs=2, space="PSUM"))
    popool = ctx.enter_context(tc.tile_pool(name="popool", bufs=2, space="PSUM"))

    # ---- constants / persistent tensors ----
    ident = consts.tile([128, 128], f32)
    make_identity(nc, ident)

    xt = consts.tile([128, NKT, S], f32r)           # x^T
    qt = consts.tile([128, 2, S], f32r)             # Q^T: slot0=[A;B], slot1=[C;-]
    kt_ = consts.tile([128, 2, S], f32r)            # K^T likewise
    v_sb = consts.tile([128, NST, HLOC, 72], f32r)  # V natural + ones col at 64
    attnT_AB = consts.tile([128, S], f32r)          # normalized attn^T heads A,B
    attnT_C = consts.tile([64, S], f32r)            # head C

    w_qsb = consts.tile([128, NKT, 192], f32r)
    w_ksb = consts.tile([128, NKT, 192], f32r)
    w_vsb = consts.tile([128, NKT, 256], f32r)
    w_osb = consts.tile([128, 2, D], f32r)
    bq1 = consts.tile([128, 1], f32)
    bq2 = consts.tile([64, 1], f32)
    bk1 = consts.tile([128, 1], f32)
    bk2 = consts.tile([64, 1], f32)
    bv_bc = consts.tile([128, 192], f32)

    # ---- weight / bias loads (outside any timing loop) ----
    nc.sync.dma_start(out=w_qsb, in_=wq.rearrange("(t p) c -> p t c", p=128))
    nc.sync.dma_start(out=w_ksb, in_=wk.rearrange("(t p) c -> p t c", p=128))
    wv_r = wv.rearrange("(t p) c -> p t c", p=128)
    nc.sync.dma_start(out=w_vsb[:, :, 0:192], in_=wv_r)
    nc.sync.dma_start(out=w_vsb[:, :, 192:256], in_=wv_r[:, :, 0:64])
    nc.sync.dma_start(out=w_osb[:, 0, :], in_=wo[0:128, :])
    nc.sync.dma_start(out=w_osb[0:64, 1, :], in_=wo[128:192, :])
    nc.sync.dma_start(out=bq1, in_=bq[0:128].rearrange("(p o) -> p o", o=1))
    nc.sync.dma_start(out=bq2, in_=bq[128:192].rearrange("(p o) -> p o", o=1))
    nc.sync.dma_start(out=bk1, in_=bk[0:128].rearrange("(p o) -> p o", o=1))
    nc.sync.dma_start(out=bk2, in_=bk[128:192].rearrange("(p o) -> p o", o=1))
    bv_b = bass.AP(tensor=bv.tensor, offset=bv.offset, ap=[[0, 128]] + list(bv.ap))
    nc.sync.dma_start(out=bv_bc, in_=bv_b)
    ones_f32 = consts.tile([128, NST * HLOC], f32)
    nc.vector.memset(ones_f32, 1.0)
    nc.vector.tensor_copy(
        out=v_sb[:, :, :, 64:65],
        in_=ones_f32.rearrange("p (a b c) -> p a b c", b=HLOC, c=1))

    acc_tiles = {}

    def qsl(g):
        return slice(g * 512, (g + 1) * 512)

    def emit_attn_block(qh, blk):
        """scores + exp + attn@V for k-tiles 4*blk..4*blk+3 of q-half qh."""
        for h in range(HLOC):
            if blk == 0:
                acc_tiles[(h, qh)] = accpool.tile(
                    [65, 1024], f32, name=f"acc_{h}_{qh}", tag="acc")
        kts = list(range(4 * blk, 4 * blk + 4))

        def scores_mm(dst, h, kti, c):
            slot, base = HPOS[h]
            nc.tensor.matmul(
                dst[:, c * 512:(c + 1) * 512],
                lhsT=kt_[base:base + 64, slot, kti * 128:(kti + 1) * 128],
                rhs=qt[base:base + 64, slot,
                       qh * 1024 + c * 512: qh * 1024 + (c + 1) * 512],
                start=True, stop=True)

        def attn_v(h, ptiles):
            if phase < 3:
                return
            acc = acc_tiles[(h, qh)]
            for c in range(2):
                po = popool.tile([65, 512], f32, name=f"po_{h}_{qh}_{blk}_{c}",
                                 tag="po")
                for i, (kti, p_t) in enumerate(ptiles):
                    nc.tensor.matmul(
                        po,
                        lhsT=v_sb[:, kti, h, 0:65],
                        rhs=p_t[:, c * 512:(c + 1) * 512],
                        start=(i == 0), stop=(i == 3))
                dst = acc[:, c * 512:(c + 1) * 512]
                if blk == 0:
                    nc.vector.tensor_copy(out=dst, in_=po)
                else:
                    nc.vector.tensor_tensor(out=dst, in0=dst, in1=po, op=add)

        # heads A,B: emit score matmuls c-major so the A matmul (array rows
        # 0-63) and B matmul (rows 64-127) are queue-adjacent and run
        # concurrently via PE row-group tiling.
        pt_ab = {}
        for kti in kts:
            pss = {}
            for h in (0, 1):
                pss[h] = pspool.tile([128, 1024], f32,
                                     name=f"ps_{h}_{qh}_{kti}", tag="ps")
            for c in range(2):
                for h in (0, 1):
                    scores_mm(pss[h], h, kti, c)
            for h in (0, 1):
                p_t = ppool.tile([128, 1024], f32r,
                                 name=f"p_{h}_{qh}_{kti}", tag="p")
                nc.scalar.activation(out=p_t, in_=pss[h], func=Exp)
                pt_ab[(h, kti)] = p_t
        for h in (0, 1):
            attn_v(h, [(kti, pt_ab[(h, kti)]) for kti in kts])
        # head C solo
        ptc = []
        for kti in kts:
            ps = pspool.tile([128, 1024], f32, name=f"ps_2_{qh}_{kti}", tag="ps")
            for c in range(2):
                scores_mm(ps, 2, kti, c)
            p_t = ppool.tile([128, 1024], f32r, name=f"p_2_{qh}_{kti}", tag="p")
            nc.scalar.activation(out=p_t, in_=ps, func=Exp)
            ptc.append((kti, p_t))
        attn_v(2, ptc)

    def finish_qh(qh):
        if phase < 3:
            return
        qhs = slice(qh * 1024, (qh + 1) * 1024)
        for h in range(HLOC):
            acc = acc_tiles.pop((h, qh))
            r_t = rpool.tile([1, 1024], f32, name=f"r_{h}_{qh}", tag="r")
            nc.vector.reciprocal(out=r_t, in_=acc[64:65, :])
            b_t = bpool.tile([64, 1024], f32, name=f"b_{h}_{qh}", tag="b")
            nc.gpsimd.partition_broadcast(b_t, r_t)
            if h == 0:
                dst = attnT_AB[0:64, qhs]
            elif h == 1:
                dst = attnT_AB[64:128, qhs]
            else:
                dst = attnT_C[0:64, qhs]
            nc.vector.tensor_tensor(out=dst, in0=acc[0:64, :], in1=b_t, op=mult)
        for sti in range(qh * 8, qh * 8 + 8):
            if phase < 4:
                break
            ssl = slice(sti * 128, (sti + 1) * 128)
            o_t = opool.tile([128, D], f32, name=f"o_{sti}", tag="o")
            for e in range(2):
                esl = slice(e * 384, (e + 1) * 384)
                pw = pab.tile([128, 512], f32, tag="mm", name=f"pw_{sti}_{e}")
                nc.tensor.matmul(pw[:, 0:384], lhsT=attnT_AB[:, ssl],
                                 rhs=w_osb[:, 0, esl],
                                 start=True, stop=False)
                nc.tensor.matmul(pw[:, 0:384],
                                 lhsT=attnT_C[0:64, ssl],
                                 rhs=w_osb[0:64, 1, esl],
                                 start=False, stop=True)
                nc.vector.tensor_copy(out=o_t[:, esl], in_=pw[:, 0:384])
            nc.sync.dma_start(out=out_d[ssl, :], in_=o_t)

    def body():
        # main pipeline over s-tile groups of 4 (one q-chunk of 512 each)
        for g in range(4):
            xg = []
            for j in range(4):
                st = 4 * g + j
                x_t = xpool.tile([128, D], f32, name=f"x_{st}", tag="x")
                nc.sync.dma_start(out=x_t, in_=xb[st * 128:(st + 1) * 128, :])
                xg.append(x_t)
            for dt in range(NKT):
                pt = pab.tile([128, 512], f32, tag="mm", name=f"pt_{g}_{dt}")
                for j in range(4):
                    nc.tensor.transpose(pt[:, j * 128:(j + 1) * 128],
                                        xg[j][:, dt * 128:(dt + 1) * 128], ident)
                nc.vector.tensor_copy(out=xt[:, dt, qsl(g)], in_=pt)
            # Q/K projections for q-chunk g
            for dst, wsb, b1, b2, sc in ((qt, w_qsb, bq1, bq2, 0.125),
                                         (kt_, w_ksb, bk1, bk2, None)):
                pp = pab.tile([128, 512], f32, tag="mm", name=f"pp_{g}")
                for kti in range(NKT):
                    nc.tensor.matmul(pp, lhsT=wsb[:, kti, 0:128],
                                     rhs=xt[:, kti, qsl(g)],
                                     start=(kti == 0), stop=(kti == NKT - 1))
                if sc is None:
                    nc.vector.tensor_scalar_add(dst[:, 0, qsl(g)], pp, b1)
                else:
                    nc.vector.tensor_scalar(dst[:, 0, qsl(g)], pp, b1, sc,
                                            add, mult)
                pp2 = pab.tile([128, 512], f32, tag="mm", name=f"pp2_{g}")
                for kti in range(NKT):
                    nc.tensor.matmul(pp2[0:64, :],
                                     lhsT=wsb[:, kti, 128:192],
                                     rhs=xt[:, kti, qsl(g)],
                                     start=(kti == 0), stop=(kti == NKT - 1))
                if sc is None:
                    nc.vector.tensor_scalar_add(dst[0:64, 1, qsl(g)],
                                                pp2[0:64, :], b2)
                else:
                    nc.vector.tensor_scalar(dst[0:64, 1, qsl(g)], pp2[0:64, :],
                                            b2, sc, add, mult)
            # V projection for s-tiles in group g
            for j in range(4):
                st = 4 * g + j
                pv = pab.tile([128, 512], f32, tag="mm", name=f"pv_{st}")
                for kti in range(NKT):
                    nc.tensor.matmul(pv[:, 0:256],
                                     lhsT=xt[:, kti,
                                             st * 128:(st + 1) * 128],
                                     rhs=w_vsb[:, kti, :],
                                     start=(kti == 0), stop=(kti == NKT - 1))
                nc.vector.tensor_tensor(
                    out=v_sb[:, st, :, 0:64],
                    in0=pv[:, 0:192].rearrange("p (h d) -> p h d", h=3),
                    in1=bv_bc.rearrange("p (h d) -> p h d", h=3),
                    op=add)
            # attention work unlocked by this group
            if phase < 2:
                continue
            if g == 1:
                emit_attn_block(0, 0)
                emit_attn_block(0, 1)
            elif g == 2:
                emit_attn_block(0, 2)
            elif g == 3:
                emit_attn_block(0, 3)
                finish_qh(0)
                for blk in range(4):
                    emit_attn_block(1, blk)
                finish_qh(1)

    if loop_n is None:
        body()
    else:
        with tc.For_i(0, loop_n, 1):
            body()


def _build(loop_n=None, phase=4):
    from contextlib import ExitStack

    import concourse.bacc as bacc
    import concourse.mybir as mybir
    import concourse.tile as tile

    f32 = mybir.dt.float32
    f32r = mybir.dt.float32r
    nc = bacc.Bacc("TRN2", target_bir_lowering=False, debug=False, num_devices=8)
    dram = {
        "xb": nc.dram_tensor("xb", [S, D], f32, kind="ExternalInput").ap(),
        "wq": nc.dram_tensor("wq", [D, 192], f32r, kind="ExternalInput").ap(),
        "wk": nc.dram_tensor("wk", [D, 192], f32r, kind="ExternalInput").ap(),
        "wv": nc.dram_tensor("wv", [D, 192], f32r, kind="ExternalInput").ap(),
        "wo": nc.dram_tensor("wo", [192, D], f32r, kind="ExternalInput").ap(),
        "bq": nc.dram_tensor("bq", [192], f32, kind="ExternalInput").ap(),
        "bk": nc.dram_tensor("bk", [192], f32, kind="ExternalInput").ap(),
        "bv": nc.dram_tensor("bv", [192], f32, kind="ExternalInput").ap(),
        "out": nc.dram_tensor("out", [S, D], f32, kind="ExternalOutput").ap(),
    }
    with tile.TileContext(nc) as tc:
        with ExitStack() as ctx:
            _emit(nc, tc, ctx, dram, loop_n=loop_n, phase=phase)
    nc.compile()
    return nc


def _get_nc():
    if "nc" not in _CACHE:
        _CACHE["nc"] = _build()
    return _CACHE["nc"]


def _shard(inputs):
    x = np.asarray(inputs["x"], np.float32)
    Wq = np.asarray(inputs["Wq"], np.float32)
    Wk = np.asarray(inputs["Wk"], np.float32)
    Wv = np.asarray(inputs["Wv"], np.float32)
    Wo = np.asarray(inputs["Wo"], np.float32)
    bq = np.asarray(inputs["bq"], np.float32)
    bk = np.asarray(inputs["bk"], np.float32)
    bv = np.asarray(inputs["bv"], np.float32)
    in_maps = []
    for c in range(8):
        b, g = divmod(c, 4)
        o = 192 * g
        in_maps.append({
            "xb": np.ascontiguousarray(x[b]),
            "wq": np.ascontiguousarray(Wq[:, o:o + 192]),
            "wk": np.ascontiguousarray(Wk[:, o:o + 192]),
            "wv": np.ascontiguousarray(Wv[:, o:o + 192]),
            "wo": np.ascontiguousarray(Wo[o:o + 192, :]),
            "bq": np.ascontiguousarray(bq[o:o + 192]),
            "bk": np.ascontiguousarray(bk[o:o + 192]),
            "bv": np.ascontiguousarray(bv[o:o + 192]),
        })
    return in_maps


def kernel(x, Wq, bq, Wk, bk, Wv, bv, Wo, bo):
    from concourse.bass_utils import run_bass_kernel_spmd

    nc = _get_nc()
    in_maps = _shard(dict(x=x, Wq=Wq, Wk=Wk, Wv=Wv, Wo=Wo,
                          bq=bq, bk=bk, bv=bv))
    res = run_bass_kernel_spmd(nc, in_maps, core_ids=list(range(8)))
    out = np.zeros((2, S, D), np.float32)
    for c in range(8):
        out[c // 4] += res.results[c]["out"]
    out += np.asarray(bo, np.float32)
    return out


# revision 13
# speedup vs baseline: 2.6448x; 1.0170x over previous
"""Multi-head self-attention (no causal mask) on 8 Trainium2 NeuronCores.

Problem: B=2, S=2048, D=768, H=12 heads (head_dim 64), fp32.
Sharding: batch x head-group. Core c handles batch c//4 and heads
3*(c%4) .. 3*(c%4)+2 (Megatron column-parallel QKV, row-parallel Wo).
Each core computes a partial [2048, 768] output (its heads' contribution
through Wo); the host sums the 4 partials per batch and adds bo.

Per-core kernel outline (all fp32):
  - load x[b] [2048,768]; transpose on PE to x^T [768,2048] in SBUF
  - Q^T/K^T per head in [64, 2048] layout (scale 1/8 + bias folded in),
    V in natural [2048, 64] layout with a ones-column appended
  - per (head, q-half, k-tile): scores^T = K^T.T @ Q^T chunks -> PSUM,
    exp on ScalarE -> p^T in SBUF, attn@V accumulates [V|1].T @ p^T
    giving both the unnormalized output and the softmax denominators
  - normalize by broadcasted reciprocal sums, out-project with Wo
"""

import numpy as np

_CACHE = {}

S = 2048
D = 768
HLOC = 3          # heads per core
NKT = 6           # 768 / 128 d-tiles
NST = 16          # 2048 / 128 s-tiles
# head -> (slot, base partition) in the Q^T/K^T storage tiles
HPOS = [(0, 0), (0, 64), (1, 0)]


def _emit(nc, tc, ctx, dram, loop_n=None, phase=4):
    import concourse.bass as bass
    import concourse.mybir as mybir
    from concourse.masks import make_identity

    f32 = mybir.dt.float32
    f32r = mybir.dt.float32r
    add = mybir.AluOpType.add
    mult = mybir.AluOpType.mult
    Exp = mybir.ActivationFunctionType.Exp

    xb, wq, wk, wv, wo, bq, bk, bv, out_d = (
        dram["xb"], dram["wq"], dram["wk"], dram["wv"], dram["wo"],
        dram["bq"], dram["bk"], dram["bv"], dram["out"],
    )

    consts = ctx.enter_context(tc.tile_pool(name="consts", bufs=1))
    xpool = ctx.enter_context(tc.tile_pool(name="xpool", bufs=5))
    ppool = ctx.enter_context(tc.tile_pool(name="ppool", bufs=8))
    accpool = ctx.enter_context(tc.tile_pool(name="accpool", bufs=3))
    rpool = ctx.enter_context(tc.tile_pool(name="rpool", bufs=1))
    bpool = ctx.enter_context(tc.tile_pool(name="bpool", bufs=1))
    opool = ctx.enter_context(tc.tile_pool(name="opool", bufs=2))
    pab = ctx.enter_context(tc.tile_pool(name="pab", bufs=2, space="PSUM"))
    pspool = ctx.enter_context(tc.tile_pool(name="pspool", bufs=2, space="PSUM"))
    popool = ctx.enter_context(tc.tile_pool(name="popool", bufs=2, space="PSUM"))

    # ---- constants / persistent tensors ----
    ident = consts.tile([128, 128], f32)
    make_identity(nc, ident)

    xt = consts.tile([128, NKT, S], f32r)           # x^T
    qt = consts.tile([128, 2, S], f32r)             # Q^T: slot0=[A;B], slot1=[C;-]
    kt_ = consts.tile([128, 2, S], f32r)            # K^T likewise
    v_sb = consts.tile([128, NST, HLOC, 72], f32r)  # V natural + ones col at 64
    attnT_AB = consts.tile([128, S], f32r)          # normalized attn^T heads A,B
    attnT_C = consts.tile([64, S], f32r)            # head C

    w_qsb = consts.tile([128, NKT, 128], f32r)
    w_ksb = consts.tile([128, NKT, 128], f32r)
    w_qkc = consts.tile([128, NKT, 128], f32r)  # [Wq_C | Wk_C]
    w_vsb = consts.tile([128, NKT, 256], f32r)
    w_osb = consts.tile([128, 2, D], f32r)
    bq1 = consts.tile([128, 1], f32)
    bq2 = consts.tile([64, 1], f32)
    bk1 = consts.tile([128, 1], f32)
    bk2 = consts.tile([64, 1], f32)
    bkC = consts.tile([128, 1], f32)
    bv_bc = consts.tile([128, 192], f32)

    # ---- weight / bias loads (outside any timing loop) ----
    wq_r = wq.rearrange("(t p) c -> p t c", p=128)
    wk_r = wk.rearrange("(t p) c -> p t c", p=128)
    nc.sync.dma_start(out=w_qsb, in_=wq_r[:, :, 0:128])
    nc.sync.dma_start(out=w_ksb, in_=wk_r[:, :, 0:128])
    nc.sync.dma_start(out=w_qkc[:, :, 0:64], in_=wq_r[:, :, 128:192])
    nc.sync.dma_start(out=w_qkc[:, :, 64:128], in_=wk_r[:, :, 128:192])
    wv_r = wv.rearrange("(t p) c -> p t c", p=128)
    nc.sync.dma_start(out=w_vsb[:, :, 0:192], in_=wv_r)
    nc.sync.dma_start(out=w_vsb[:, :, 192:256], in_=wv_r[:, :, 0:64])
    nc.sync.dma_start(out=w_osb[:, 0, :], in_=wo[0:128, :])
    nc.sync.dma_start(out=w_osb[0:64, 1, :], in_=wo[128:192, :])
    nc.sync.dma_start(out=bq1, in_=bq[0:128].rearrange("(p o) -> p o", o=1))
    nc.sync.dma_start(out=bq2, in_=bq[128:192].rearrange("(p o) -> p o", o=1))
    nc.sync.dma_start(out=bk1, in_=bk[0:128].rearrange("(p o) -> p o", o=1))
    nc.sync.dma_start(out=bk2, in_=bk[128:192].rearrange("(p o) -> p o", o=1))
    nc.sync.dma_start(out=bkC[64:128, :],
                      in_=bk[128:192].rearrange("(p o) -> p o", o=1))
    bv_b = bass.AP(tensor=bv.tensor, offset=bv.offset, ap=[[0, 128]] + list(bv.ap))
    nc.sync.dma_start(out=bv_bc, in_=bv_b)
    ones_f32 = consts.tile([128, NST * HLOC], f32)
    nc.vector.memset(ones_f32, 1.0)
    nc.vector.tensor_copy(
        out=v_sb[:, :, :, 64:65],
        in_=ones_f32.rearrange("p (a b c) -> p a b c", b=HLOC, c=1))

    acc_tiles = {}

    def qsl(g):
        return slice(g * 512, (g + 1) * 512)

    def emit_attn_block(qh, blk):
        """scores + exp + attn@V for k-tiles 4*blk..4*blk+3 of q-half qh."""
        for h in range(HLOC):
            if blk == 0:
                acc_tiles[(h, qh)] = accpool.tile(
                    [65, 1024], f32, name=f"acc_{h}_{qh}", tag="acc")
        kts = list(range(4 * blk, 4 * blk + 4))

        def scores_mm(dst, h, kti, c):
            slot, base = HPOS[h]
            nc.tensor.matmul(
                dst[:, c * 512:(c + 1) * 512],
                lhsT=kt_[base:base + 64, slot, kti * 128:(kti + 1) * 128],
                rhs=qt[base:base + 64, slot,
                       qh * 1024 + c * 512: qh * 1024 + (c + 1) * 512],
                start=True, stop=True)

        def attn_v(h, ptiles):
            if phase < 3:
                return
            acc = acc_tiles[(h, qh)]
            for c in range(2):
                po = popool.tile([65, 512], f32, name=f"po_{h}_{qh}_{blk}_{c}",
                                 tag="po")
                for i, (kti, p_t) in enumerate(ptiles):
                    nc.tensor.matmul(
                        po,
                        lhsT=v_sb[:, kti, h, 0:65],
                        rhs=p_t[:, c * 512:(c + 1) * 512],
                        start=(i == 0), stop=(i == 3))
                dst = acc[:, c * 512:(c + 1) * 512]
                if blk == 0:
                    nc.vector.tensor_copy(out=dst, in_=po)
                else:
                    nc.vector.tensor_tensor(out=dst, in0=dst, in1=po, op=add)

        # heads A,B: emit score matmuls c-major so the A matmul (array rows
        # 0-63) and B matmul (rows 64-127) are queue-adjacent and run
        # concurrently via PE row-group tiling.
        pt_ab = {}
        for kti in kts:
            pss = {}
            for h in (0, 1):
                pss[h] = pspool.tile([128, 1024], f32,
                                     name=f"ps_{h}_{qh}_{kti}", tag="ps")
            for c in range(2):
                for h in (0, 1):
                    scores_mm(pss[h], h, kti, c)
            for h in (0, 1):
                p_t = ppool.tile([128, 1024], f32r,
                                 name=f"p_{h}_{qh}_{kti}", tag="p")
                nc.scalar.activation(out=p_t, in_=pss[h], func=Exp)
                pt_ab[(h, kti)] = p_t
        for h in (0, 1):
            attn_v(h, [(kti, pt_ab[(h, kti)]) for kti in kts])
        # head C: pair consecutive k-tiles on PE row groups 0/64
        def scores_mm_c(dst, kti, c, base):
            nc.tensor.matmul(
                dst[:, c * 512:(c + 1) * 512],
                lhsT=kt_[base:base + 64, 1, kti * 128:(kti + 1) * 128],
                rhs=qt[base:base + 64, 1,
                       qh * 1024 + c * 512: qh * 1024 + (c + 1) * 512],
                start=True, stop=True)

        ptc = []
        for ka, kb in ((kts[0], kts[1]), (kts[2], kts[3])):
            psa = pspool.tile([128, 1024], f32, name=f"ps_2_{qh}_{ka}", tag="ps")
            psb = pspool.tile([128, 1024], f32, name=f"ps_2_{qh}_{kb}", tag="ps")
            for c in range(2):
                scores_mm_c(psa, ka, c, 0)
                scores_mm_c(psb, kb, c, 64)
            for kti, ps in ((ka, psa), (kb, psb)):
                p_t = ppool.tile([128, 1024], f32r,
                                 name=f"p_2_{qh}_{kti}", tag="p")
                nc.scalar.activation(out=p_t, in_=ps, func=Exp)
                ptc.append((kti, p_t))
        attn_v(2, ptc)

    def finish_qh(qh):
        if phase < 3:
            return
        qhs = slice(qh * 1024, (qh + 1) * 1024)
        for h in range(HLOC):
            acc = acc_tiles.pop((h, qh))
            r_t = rpool.tile([1, 1024], f32, name=f"r_{h}_{qh}", tag="r")
            nc.vector.reciprocal(out=r_t, in_=acc[64:65, :])
            b_t = bpool.tile([64, 1024], f32, name=f"b_{h}_{qh}", tag="b")
            nc.gpsimd.partition_broadcast(b_t, r_t)
            if h == 0:
                dst = attnT_AB[0:64, qhs]
            elif h == 1:
                dst = attnT_AB[64:128, qhs]
            else:
                dst = attnT_C[0:64, qhs]
            nc.vector.tensor_tensor(out=dst, in0=acc[0:64, :], in1=b_t, op=mult)
        for sti in range(qh * 8, qh * 8 + 8):
            if phase < 4:
                break
            ssl = slice(sti * 128, (sti + 1) * 128)
            o_t = opool.tile([128, D], f32, name=f"o_{sti}", tag="o")
            for e in range(2):
                esl = slice(e * 384, (e + 1) * 384)
                pw = pab.tile([128, 512], f32, tag="mm", name=f"pw_{sti}_{e}")
                nc.tensor.matmul(pw[:, 0:384], lhsT=attnT_AB[:, ssl],
                                 rhs=w_osb[:, 0, esl],
                                 start=True, stop=False)
                nc.tensor.matmul(pw[:, 0:384],
                                 lhsT=attnT_C[0:64, ssl],
                                 rhs=w_osb[0:64, 1, esl],
                                 start=False, stop=True)
                nc.vector.tensor_copy(out=o_t[:, esl], in_=pw[:, 0:384])
            nc.sync.dma_start(out=out_d[ssl, :], in_=o_t)

    def body():
        # main pipeline over s-tile groups of 4 (one q-chunk of 512 each)
        for g in range(4):
            xg = []
            for j in range(4):
                st = 4 * g + j
                x_t = xpool.tile([128, D], f32, name=f"x_{st}", tag="x")
                nc.sync.dma_start(out=x_t, in_=xb[st * 128:(st + 1) * 128, :])
                xg.append(x_t)
            for dt in range(NKT):
                pt = pab.tile([128, 512], f32, tag="mm", name=f"pt_{g}_{dt}")
                for j in range(4):
                    nc.tensor.transpose(pt[:, j * 128:(j + 1) * 128],
                                        xg[j][:, dt * 128:(dt + 1) * 128], ident)
                nc.vector.tensor_copy(out=xt[:, dt, qsl(g)], in_=pt)
            # Q/K projections for q-chunk g: heads A,B (M=128 per matrix)
            for dst, wsb, b1, sc in ((qt, w_qsb, bq1, 0.125),
                                     (kt_, w_ksb, bk1, None)):
                pp = pab.tile([128, 512], f32, tag="mm", name=f"pp_{g}")
                for kti in range(NKT):
                    nc.tensor.matmul(pp, lhsT=wsb[:, kti, :],
                                     rhs=xt[:, kti, qsl(g)],
                                     start=(kti == 0), stop=(kti == NKT - 1))
                if sc is None:
                    nc.vector.tensor_scalar_add(dst[:, 0, qsl(g)], pp, b1)
                else:
                    nc.vector.tensor_scalar(dst[:, 0, qsl(g)], pp, b1, sc,
                                            add, mult)
            # head C: merged [Q_C | K_C] group (M=128), then duplicate each
            # half to the other partition range so C score matmuls can pair
            # on PE row groups 0/64.
            pp2 = pab.tile([128, 512], f32, tag="mm", name=f"pp2_{g}")
            for kti in range(NKT):
                nc.tensor.matmul(pp2, lhsT=w_qkc[:, kti, :],
                                 rhs=xt[:, kti, qsl(g)],
                                 start=(kti == 0), stop=(kti == NKT - 1))
            nc.vector.tensor_scalar(qt[0:64, 1, qsl(g)], pp2[0:64, :],
                                    bq2, 0.125, add, mult)
            nc.vector.tensor_scalar_add(kt_[64:128, 1, qsl(g)],
                                        pp2[64:128, :], bkC[64:128, :])
            nc.vector.tensor_copy(out=qt[64:128, 1, qsl(g)],
                                  in_=qt[0:64, 1, qsl(g)])
            nc.vector.tensor_copy(out=kt_[0:64, 1, qsl(g)],
                                  in_=kt_[64:128, 1, qsl(g)])
            # V projection for s-tiles in group g
            for j in range(4):
                st = 4 * g + j
                pv = pab.tile([128, 512], f32, tag="mm", name=f"pv_{st}")
                for kti in range(NKT):
                    nc.tensor.matmul(pv[:, 0:256],
                                     lhsT=xt[:, kti,
                                             st * 128:(st + 1) * 128],
                                     rhs=w_vsb[:, kti, :],
                                     start=(kti == 0), stop=(kti == NKT - 1))
                nc.vector.tensor_tensor(
                    out=v_sb[:, st, :, 0:64],
                    in0=pv[:, 0:192].rearrange("p (h d) -> p h d", h=3),
                    in1=bv_bc.rearrange("p (h d) -> p h d", h=3),
                    op=add)
            # attention work unlocked by this group
            if phase < 2:
                continue
            if g == 1:
                emit_attn_block(0, 0)
                emit_attn_block(0, 1)
            elif g == 2:
                emit_attn_block(0, 2)
            elif g == 3:
                emit_attn_block(0, 3)
                finish_qh(0)
                for blk in range(4):
                    emit_attn_block(1, blk)
                finish_qh(1)

    if loop_n is None:
        body()
    else:
        with tc.For_i(0, loop_n, 1):
            body()


def _build(loop_n=None, phase=4):
    from contextlib import ExitStack

    import concourse.bacc as bacc
    import concourse.mybir as mybir
    import concourse.tile as tile

    f32 = mybir.dt.float32
    f32r = mybir.dt.float32r
    nc = bacc.Bacc("TRN2", target_bir_lowering=False, debug=False, num_devices=8)
    dram = {
        "xb": nc.dram_tensor("xb", [S, D], f32, kind="ExternalInput").ap(),
        "wq": nc.dram_tensor("wq", [D, 192], f32r, kind="ExternalInput").ap(),
        "wk": nc.dram_tensor("wk", [D, 192], f32r, kind="ExternalInput").ap(),
        "wv": nc.dram_tensor("wv", [D, 192], f32r, kind="ExternalInput").ap(),
        "wo": nc.dram_tensor("wo", [192, D], f32r, kind="ExternalInput").ap(),
        "bq": nc.dram_tensor("bq", [192], f32, kind="ExternalInput").ap(),
        "bk": nc.dram_tensor("bk", [192], f32, kind="ExternalInput").ap(),
        "bv": nc.dram_tensor("bv", [192], f32, kind="ExternalInput").ap(),
        "out": nc.dram_tensor("out", [S, D], f32, kind="ExternalOutput").ap(),
    }
    with tile.TileContext(nc) as tc:
        with ExitStack() as ctx:
            _emit(nc, tc, ctx, dram, loop_n=loop_n, phase=phase)
    nc.compile()
    return nc


def _get_nc():
    if "nc" not in _CACHE:
        _CACHE["nc"] = _build()
    return _CACHE["nc"]


def _shard(inputs):
    x = np.asarray(inputs["x"], np.float32)
    Wq = np.asarray(inputs["Wq"], np.float32)
    Wk = np.asarray(inputs["Wk"], np.float32)
    Wv = np.asarray(inputs["Wv"], np.float32)
    Wo = np.asarray(inputs["Wo"], np.float32)
    bq = np.asarray(inputs["bq"], np.float32)
    bk = np.asarray(inputs["bk"], np.float32)
    bv = np.asarray(inputs["bv"], np.float32)
    in_maps = []
    for c in range(8):
        b, g = divmod(c, 4)
        o = 192 * g
        in_maps.append({
            "xb": np.ascontiguousarray(x[b]),
            "wq": np.ascontiguousarray(Wq[:, o:o + 192]),
            "wk": np.ascontiguousarray(Wk[:, o:o + 192]),
            "wv": np.ascontiguousarray(Wv[:, o:o + 192]),
            "wo": np.ascontiguousarray(Wo[o:o + 192, :]),
            "bq": np.ascontiguousarray(bq[o:o + 192]),
            "bk": np.ascontiguousarray(bk[o:o + 192]),
            "bv": np.ascontiguousarray(bv[o:o + 192]),
        })
    return in_maps


def kernel(x, Wq, bq, Wk, bk, Wv, bv, Wo, bo):
    from concourse.bass_utils import run_bass_kernel_spmd

    nc = _get_nc()
    in_maps = _shard(dict(x=x, Wq=Wq, Wk=Wk, Wv=Wv, Wo=Wo,
                          bq=bq, bk=bk, bv=bv))
    res = run_bass_kernel_spmd(nc, in_maps, core_ids=list(range(8)))
    out = np.zeros((2, S, D), np.float32)
    for c in range(8):
        out[c // 4] += res.results[c]["out"]
    out += np.asarray(bo, np.float32)
    return out
